# revision 17
# baseline (speedup 1.0000x reference)
"""Sparse-attention Bass/Tile kernel for nn_Attention_53558242181469.

SPMD over 8 NeuronCores: the 48 heads (4 branches x 12 sub-heads) are split
6-per-core (each core owns 6 contiguous sub-heads of one branch), so Wq/Wk/WO
are row-sharded too.  Per-core, everything runs as one Bass/Tile program:

  * Q/K projections in transposed layout (d on partitions) off PE-transposed
    A^T/X^T; q's rmsnorm is folded into the per-row exp scale
    (1/sqrt(ssq+64*eps) absorbs both rsqrt(mean+eps) and dh^-0.5).
  * BiasedWedge as a single 64x64 matmul (I + S^T, host-permuted), RoPE as
    2 DVE muls against stacked [cos;sin]/[sin;cos] tables + 2 GpSimd
    cross-half add/subs (head dims pre-permuted to even/odd halves via the
    weight rows).  Q path and q/k rotated vectors run in bf16 (PE 4x).
  * scores = qr^T @ kr per 128-row block (causal: only lower-triangular
    column blocks computed); exp on the Scalar engine with accum_out giving
    the softmax denominator for free; the sink is exp(sink) host-folded.
  * top-12 via DVE max8/match_replace: top-8 of e, knock out, top-8 again;
    the 12th-largest value becomes a per-row threshold and
    masked = e * (e >= t12) * (1/(13*denom)) in two fused DVE ops.
  * masked (bf16) is PE-transposed per 128x128 block and contracted with the
    vanilla keys (bf16); the +kv/13 term is added exactly in fp32 afterward.
  * V_net MLP in transposed layout (fp32); rmsnorm-over-256 via ones-matmul
    + rank-1 broadcast; h*sigmoid(c*h) as Silu(c*h) with proj_w pre-scaled.
  * per-branch W_O per row block (ctx stacked 6 heads -> 384 rows), WO_b
    mean/8 via a rank-1 matmul; ReduceScatter over the 8 cores sums the
    branches; each core emits its 256-row slice of the (2048,768) output.

Inputs are packed into 3 device tensors (AX activations, WB32/WB16 weight
blobs) to minimize per-dispatch argument overhead.
"""

import numpy as np
import ml_dtypes

import concourse.mybir as mybir
import concourse.tile as tile
from concourse import bacc
from concourse.bass_utils import run_bass_kernel_spmd
from concourse.masks import make_causal_mask, make_identity

F32 = mybir.dt.float32
BF16 = mybir.dt.bfloat16
AF = mybir.ActivationFunctionType
ALU = mybir.AluOpType

D_MODEL, N_HEAD, N_BR = 768, 12, 4
DH = 64
H_TOT = 48
K_RETR = 12
MLP_SCALE = float(np.pi / np.sqrt(3.0))
N_CORES, HPC = 8, 6
B, T = 2, 1024
NRB = T // 128
EPS = float(np.finfo(np.float32).eps)
PERM = np.concatenate([np.arange(0, DH, 2), np.arange(1, DH, 2)])
NEG = -1.0e30

# (name, shape) entries of the fp32 weight blob, in pack order
WSPEC32 = [
    ("WkT", (D_MODEL, HPC * DH)),
    ("WEDGE", (DH, HPC * DH)),
    ("C2", (DH, T)),
    ("S2", (DH, T)),
    ("ESINK", (128, HPC)),
    ("VNS", (1, HPC * DH)),
    ("FA", (DH + 1, 256)),
    ("PTP", (128, 128)),
    ("PB", (DH, 1)),
    ("WOr", (HPC * DH, D_MODEL)),
    ("WOB8", (1, D_MODEL)),
    ("QB", (DH, HPC)),
    ("KB", (DH, HPC)),
]
WSPEC16 = [
    ("WqT16", (D_MODEL, HPC * DH)),
]


def _blob_views(handle, spec):
    views, off = {}, 0
    for name, (r, c) in spec:
        n = r * c
        views[name] = handle[off:off + n].rearrange("(r c) -> r c", c=c)
        off += n
    return views


def _blob_size(spec):
    return sum(r * c for _, (r, c) in spec)


def _emit(tc, io, single_core=False):
    nc = tc.nc

    cpool = tc.alloc_tile_pool(name="const", bufs=1)
    dpool = tc.alloc_tile_pool(name="dram", bufs=1, space="DRAM")
    psum = tc.alloc_tile_pool(name="psum", bufs=2, space="PSUM")
    sb1 = tc.alloc_tile_pool(name="sb1", bufs=1)
    sb2 = tc.alloc_tile_pool(name="sb2", bufs=2)
    sb3 = tc.alloc_tile_pool(name="sb3", bufs=3)

    w32 = _blob_views(io["WB32"], WSPEC32)
    w16 = _blob_views(io["WB16"], WSPEC16)
    AX = io["AX"]

    # ---------------- constants -> SBUF ----------------
    wqt = [cpool.tile([128, HPC * DH], BF16, name=f"wqt{ci}") for ci in range(6)]
    wkt = [cpool.tile([128, HPC * DH], F32, name=f"wkt{ci}") for ci in range(6)]
    for ci in range(6):
        nc.sync.dma_start(wqt[ci], w16["WqT16"][ci * 128:(ci + 1) * 128, :])
        nc.sync.dma_start(wkt[ci], w32["WkT"][ci * 128:(ci + 1) * 128, :])
    qb_t = cpool.tile([DH, HPC], F32, name="qb_t")
    kb_t = cpool.tile([DH, HPC], F32, name="kb_t")
    nc.sync.dma_start(qb_t, w32["QB"])
    nc.sync.dma_start(kb_t, w32["KB"])
    wedge_t = cpool.tile([DH, HPC * DH], F32, name="wedge_t")
    nc.sync.dma_start(wedge_t, w32["WEDGE"])
    c2_t = cpool.tile([DH, T], F32, name="c2_t")
    s2_t = cpool.tile([DH, T], F32, name="s2_t")
    nc.sync.dma_start(c2_t, w32["C2"])
    nc.sync.dma_start(s2_t, w32["S2"])
    esink_t = cpool.tile([128, HPC], F32, name="esink_t")
    nc.sync.dma_start(esink_t, w32["ESINK"])
    vns_t = cpool.tile([1, HPC * DH], F32, name="vns_t")
    nc.sync.dma_start(vns_t, w32["VNS"])
    fa_t = cpool.tile([DH + 1, 256], F32, name="fa_t")
    nc.sync.dma_start(fa_t, w32["FA"])
    ptp_t = cpool.tile([128, 128], F32, name="ptp_t")
    nc.sync.dma_start(ptp_t, w32["PTP"])
    pb_t = cpool.tile([DH, 1], F32, name="pb_t")
    nc.sync.dma_start(pb_t, w32["PB"])
    wo_t = [cpool.tile([128, D_MODEL], F32, name=f"wo{ci}") for ci in range(3)]
    for ci in range(3):
        nc.sync.dma_start(wo_t[ci], w32["WOr"][ci * 128:(ci + 1) * 128, :])
    wob8_t = cpool.tile([1, D_MODEL], F32, name="wob8_t")
    nc.sync.dma_start(wob8_t, w32["WOB8"])

    ident = cpool.tile([128, 128], F32, name="ident")
    make_identity(nc, ident)
    identb = cpool.tile([128, 128], BF16, name="identb")
    make_identity(nc, identb)
    cmask = cpool.tile([128, 128], F32, name="cmask")
    make_causal_mask(nc, cmask, mask_val=NEG)
    ones_row = cpool.tile([1, 128], F32, name="ones_row")
    nc.gpsimd.memset(ones_row, 1.0)
    ones_col = cpool.tile([128, 1], F32, name="ones_col")
    nc.gpsimd.memset(ones_col, 1.0)
    c64eps = cpool.tile([128, 1], F32, name="c64eps")
    nc.gpsimd.memset(c64eps, float(DH) * EPS)
    ceps = cpool.tile([1, 1], F32, name="ceps")
    nc.gpsimd.memset(ceps, EPS)

    ybounce = dpool.tile([B * T, D_MODEL], F32, name="ybounce")
    yrs = dpool.tile([B * T // N_CORES, D_MODEL], F32, name="yrs")

    # ---------------- main program ----------------
    for b in range(B):
        # A^T (bf16) / X^T (fp32) via PE transposes of DMA'd row tiles
        at = [sb1.tile([128, T], BF16, name=f"at{ci}", tag=f"at{ci}")
              for ci in range(6)]
        xt = [sb1.tile([128, T], F32, name=f"xt{ci}", tag=f"xt{ci}")
              for ci in range(6)]
        for base, dst in ((b * T, at), (B * T + b * T, xt)):
            for rt in range(NRB):
                arow = sb3.tile([128, D_MODEL], F32, name="arow", tag="arow")
                nc.sync.dma_start(arow, AX[base + rt * 128: base + (rt + 1) * 128, :])
                for ci in range(6):
                    tp = psum.tile([128, 128], F32, name="tpa", tag="sm")
                    nc.tensor.transpose(tp, arow[:, ci * 128:(ci + 1) * 128], ident)
                    if ci % 2 == 0:
                        nc.scalar.copy(dst[ci][:, rt * 128:(rt + 1) * 128], tp)
                    else:
                        nc.vector.tensor_copy(dst[ci][:, rt * 128:(rt + 1) * 128], tp)

        ctx_tiles = [sb1.tile([128, 3 * 128], F32, name=f"ctx{rb}", tag=f"ctx{rb}")
                     for rb in range(NRB)]

        for h in range(HPC):
            hs = slice(h * DH, (h + 1) * DH)
            # ---- Q projection (bf16 inputs, fp32 accum) ----
            qp = psum.tile([DH, T], F32, name="qp", tag="mm")
            for nh in range(2):
                ns = slice(nh * 512, (nh + 1) * 512)
                for ci in range(6):
                    nc.tensor.matmul(qp[:, ns], wqt[ci][:, hs], at[ci][:, ns],
                                     start=(ci == 0), stop=(ci == 5))
            q_sb = sb1.tile([DH, T], F32, name="q_sb", tag="q_sb")
            nc.scalar.activation(q_sb, qp, AF.Identity, bias=qb_t[:, h:h + 1])
            sq = sb1.tile([DH, T], F32, name="sq", tag="sq")
            nc.scalar.activation(sq, qp, AF.Square, bias=qb_t[:, h:h + 1])
            ssq_ps = psum.tile([128, NRB], F32, name="ssq_ps", tag="sm")
            for rb in range(NRB):
                nc.tensor.matmul(ssq_ps[:, rb:rb + 1],
                                 sq[:, rb * 128:(rb + 1) * 128],
                                 ones_col[0:DH, :], start=True, stop=True)
            r8 = sb2.tile([128, NRB], F32, name="r8", tag="r8")
            nc.scalar.activation(r8, ssq_ps, AF.Sqrt, bias=c64eps)
            s8 = sb2.tile([128, NRB], F32, name="s8", tag="s8")
            nc.vector.reciprocal(s8, r8)

            # ---- wedge + rope for q (-> bf16 qr) ----
            qr = sb2.tile([DH, T], BF16, name="qr", tag="qr")
            wp = psum.tile([DH, T], F32, name="wp", tag="mm")
            for nh in range(2):
                ns = slice(nh * 512, (nh + 1) * 512)
                nc.tensor.matmul(wp[:, ns], wedge_t[:, hs], q_sb[:, ns],
                                 start=True, stop=True)
            qa = sb2.tile([32, T], F32, name="qa", tag="ropetmp")
            qb2 = sb2.tile([32, T], F32, name="qb2", tag="ropetmp")
            nc.vector.tensor_mul(qa, wp[0:32, :], c2_t[0:32, :])
            nc.vector.tensor_mul(qb2, wp[32:64, :], s2_t[0:32, :])
            nc.gpsimd.tensor_sub(qr[0:32, :], qa, qb2)
            qc = sb2.tile([32, T], F32, name="qc", tag="ropetmp")
            qd = sb2.tile([32, T], F32, name="qd", tag="ropetmp")
            nc.vector.tensor_mul(qc, wp[0:32, :], s2_t[0:32, :])
            nc.vector.tensor_mul(qd, wp[32:64, :], c2_t[0:32, :])
            nc.gpsimd.tensor_add(qr[32:64, :], qc, qd)

            # ---- K projection (fp32, vanilla keys stay exact) ----
            kp = psum.tile([DH, T], F32, name="kp", tag="mm")
            for nh in range(2):
                ns = slice(nh * 512, (nh + 1) * 512)
                for ci in range(6):
                    nc.tensor.matmul(kp[:, ns], wkt[ci][:, hs], xt[ci][:, ns],
                                     start=(ci == 0), stop=(ci == 5))
            kv_sb = sb2.tile([DH, T], F32, name="kv_sb", tag="kv_sb")
            nc.scalar.activation(kv_sb, kp, AF.Identity, bias=kb_t[:, h:h + 1])
            kv13 = sb2.tile([DH, T], F32, name="kv13", tag="kv13")
            nc.vector.tensor_scalar_mul(kv13, kv_sb, 1.0 / (K_RETR + 1.0))
            kr = sb2.tile([DH, T], BF16, name="kr", tag="kr")
            wpk = psum.tile([DH, T], F32, name="wpk", tag="mm")
            for nh in range(2):
                ns = slice(nh * 512, (nh + 1) * 512)
                nc.tensor.matmul(wpk[:, ns], wedge_t[:, hs], kv_sb[:, ns],
                                 start=True, stop=True)
            ka = sb2.tile([32, T], F32, name="ka", tag="ropetmp")
            kb2 = sb2.tile([32, T], F32, name="kb2", tag="ropetmp")
            nc.vector.tensor_mul(ka, wpk[0:32, :], c2_t[0:32, :])
            nc.vector.tensor_mul(kb2, wpk[32:64, :], s2_t[0:32, :])
            nc.gpsimd.tensor_sub(kr[0:32, :], ka, kb2)
            kc = sb2.tile([32, T], F32, name="kc", tag="ropetmp")
            kd = sb2.tile([32, T], F32, name="kd", tag="ropetmp")
            nc.vector.tensor_mul(kc, wpk[0:32, :], s2_t[0:32, :])
            nc.vector.tensor_mul(kd, wpk[32:64, :], c2_t[0:32, :])
            nc.gpsimd.tensor_add(kr[32:64, :], kc, kd)

            # ---- vanilla keys in row layout (bf16, for the marker matmul) ----
            kvrow = sb1.tile([128, NRB * DH], BF16, name="kvrow", tag="kvrow")
            for j in range(NRB):
                tpk = psum.tile([128, DH], F32, name="tpk", tag="sm")
                nc.tensor.transpose(tpk, kv_sb[:, j * 128:(j + 1) * 128],
                                    ident[0:DH, 0:DH])
                nc.scalar.copy(kvrow[:, j * DH:(j + 1) * DH], tpk)

            marker_sb = sb1.tile([DH + 1, T], F32, name="marker_sb", tag="marker")
            nc.gpsimd.memset(marker_sb[DH:DH + 1, :], 1.0)

            for rb in range(NRB):
                W = 128 * (rb + 1)
                ds = slice(rb * 128, W)
                # ---- scores (bf16 matmul, fp32 accum) ----
                sc_ps = psum.tile([128, T], F32, name="sc_ps", tag="mm")
                for n0 in range(0, W, 512):
                    nw = min(512, W - n0)
                    nc.tensor.matmul(sc_ps[:, n0:n0 + nw], qr[:, ds],
                                     kr[:, n0:n0 + nw], start=True, stop=True)
                nc.vector.tensor_add(sc_ps[:, ds], sc_ps[:, ds], cmask)
                # ---- exp + denominator ----
                e_t = sb2.tile([128, T], F32, name="e_t", tag="e_t")
                acc = sb2.tile([128, 1], F32, name="acc", tag="acc")
                nc.scalar.activation(e_t[:, 0:W], sc_ps[:, 0:W], AF.Exp,
                                     scale=s8[:, rb:rb + 1], accum_out=acc)
                denom = sb2.tile([128, 1], F32, name="denom", tag="denom")
                nc.vector.tensor_scalar(denom, acc, esink_t[:, h:h + 1], None,
                                        op0=ALU.add)
                recip = sb2.tile([128, 1], F32, name="recip", tag="recip")
                nc.vector.reciprocal(recip, denom)
                recip13 = sb2.tile([128, 1], F32, name="recip13", tag="recip13")
                nc.vector.tensor_scalar_mul(recip13, recip,
                                            1.0 / (K_RETR + 1.0))
                # ---- top-12 threshold via max8 + match_replace + max8 ----
                m8a = sb2.tile([128, 8], F32, name="m8a", tag="m8a")
                nc.vector.max(out=m8a, in_=e_t[:, 0:W])
                work = sb2.tile([128, T], F32, name="work", tag="work")
                nc.vector.match_replace(out=work[:, 0:W], in_to_replace=m8a,
                                        in_values=e_t[:, 0:W], imm_value=0.0)
                m8b = sb2.tile([128, 8], F32, name="m8b", tag="m8b")
                nc.vector.max(out=m8b, in_=work[:, 0:W])
                # mask_scaled = (e >= t12) * recip13, reusing `work`
                nc.vector.tensor_scalar(work[:, 0:W], e_t[:, 0:W],
                                        m8b[:, 3:4], recip13,
                                        op0=ALU.is_ge, op1=ALU.mult)
                masked = sb2.tile([128, T], BF16, name="masked", tag="masked")
                nc.vector.tensor_mul(masked[:, 0:W], work[:, 0:W], e_t[:, 0:W])
                # ---- probs_sink row ----
                srow_ps = psum.tile([1, 128], F32, name="srow_ps", tag="sm")
                nc.tensor.matmul(srow_ps, recip, ident, start=True, stop=True)
                reciprow = sb2.tile([1, 128], F32, name="reciprow", tag="reciprow")
                nc.scalar.copy(reciprow, srow_ps)
                # ---- marker^T = masked @ kv (bf16) + kv/13 (fp32) ----
                mk_ps = psum.tile([DH, 128], F32, name="mk_ps", tag="mk")
                for j in range(rb + 1):
                    tpm = psum.tile([128, 128], BF16, name="tpm", tag="sm")
                    nc.tensor.transpose(tpm, masked[:, j * 128:(j + 1) * 128],
                                        identb)
                    mT = sb3.tile([128, 128], BF16, name="mT", tag="mT")
                    nc.scalar.copy(mT, tpm)
                    nc.tensor.matmul(mk_ps, kvrow[:, j * DH:(j + 1) * DH], mT,
                                     start=(j == 0), stop=(j == rb))
                nc.vector.tensor_add(marker_sb[0:DH, ds], mk_ps, kv13[:, ds])
                # ---- V_net MLP (transposed, fp32) ----
                h1_ps = psum.tile([128, 256], F32, name="h1_ps", tag="sm")
                nc.tensor.matmul(h1_ps[:, 0:128], fa_t[:, 0:128],
                                 marker_sb[:, ds], start=True, stop=True)
                nc.tensor.matmul(h1_ps[:, 128:256], fa_t[:, 128:256],
                                 marker_sb[:, ds], start=True, stop=True)
                s1 = sb2.tile([128, 256], F32, name="s1", tag="s1")
                nc.scalar.activation(s1, h1_ps, AF.Copy, bias=1.0, scale=0.75)
                sqm = sb2.tile([128, 256], F32, name="sqm", tag="sqm")
                nc.scalar.activation(sqm, h1_ps, AF.Square)
                hp = sb2.tile([128, 256], F32, name="hp", tag="hp")
                nc.vector.tensor_mul(hp, sqm, s1)
                sq2 = sb2.tile([128, 256], F32, name="sq2", tag="sq2")
                nc.scalar.activation(sq2, hp, AF.Square)
                mss_ps = psum.tile([1, 128], F32, name="mss_ps", tag="sm")
                nc.tensor.matmul(mss_ps, ones_col, sq2[:, 0:128],
                                 start=True, stop=False)
                nc.tensor.matmul(mss_ps, ones_col, sq2[:, 128:256],
                                 start=False, stop=True)
                rmsrow = sb2.tile([1, 128], F32, name="rmsrow", tag="rmsrow")
                nc.scalar.activation(rmsrow, mss_ps, AF.Sqrt, bias=ceps,
                                     scale=1.0 / 256.0)
                rmscol_ps = psum.tile([128, 1], F32, name="rmscol_ps", tag="sm")
                nc.tensor.matmul(rmscol_ps, rmsrow, ones_row[0:1, 0:1],
                                 start=True, stop=True)
                invcol = sb2.tile([128, 1], F32, name="invcol", tag="invcol")
                nc.vector.reciprocal(invcol, rmscol_ps)
                invrow_ps = psum.tile([1, 128], F32, name="invrow_ps", tag="sm")
                nc.tensor.matmul(invrow_ps, invcol, ident, start=True, stop=True)
                invrow = sb2.tile([1, 128], F32, name="invrow", tag="invrow")
                nc.scalar.copy(invrow, invrow_ps)
                invbc_ps = psum.tile([128, 128], F32, name="invbc_ps", tag="sm")
                nc.tensor.matmul(invbc_ps, ones_row, invrow, start=True, stop=True)
                hn = sb2.tile([128, 256], F32, name="hn", tag="hn")
                nc.vector.tensor_mul(hn[:, 0:128], hp[:, 0:128], invbc_ps)
                nc.vector.tensor_mul(hn[:, 128:256], hp[:, 128:256], invbc_ps)
                hf = sb2.tile([128, 256], F32, name="hf", tag="hf")
                nc.scalar.activation(hf, hn, AF.Silu, scale=MLP_SCALE)
                ot_ps = psum.tile([DH, 128], F32, name="ot_ps", tag="mk")
                nc.tensor.matmul(ot_ps, ptp_t[:, 0:DH], hf[:, 0:128],
                                 start=True, stop=False)
                nc.tensor.matmul(ot_ps, ptp_t[:, DH:128], hf[:, 128:256],
                                 start=False, stop=False)
                nc.tensor.matmul(ot_ps, vns_t[0:1, hs], reciprow,
                                 start=False, stop=True)
                nc.scalar.activation(
                    ctx_tiles[rb][DH * (h % 2):DH * (h % 2) + DH,
                                  128 * (h // 2):128 * (h // 2) + 128],
                    ot_ps, AF.Identity, bias=pb_t)

        # ---- output projection + bias per row block ----
        for rb in range(NRB):
            y_ps = psum.tile([128, D_MODEL], F32, name="y_ps", tag="mm")
            for n0, nw in ((0, 512), (512, 256)):
                for ci in range(3):
                    nc.tensor.matmul(y_ps[:, n0:n0 + nw],
                                     ctx_tiles[rb][:, ci * 128:(ci + 1) * 128],
                                     wo_t[ci][:, n0:n0 + nw],
                                     start=(ci == 0), stop=False)
                nc.tensor.matmul(y_ps[:, n0:n0 + nw], ones_row,
                                 wob8_t[0:1, n0:n0 + nw], start=False, stop=True)
            y_sb = sb2.tile([128, D_MODEL], F32, name="y_sb", tag="y_sb")
            nc.vector.tensor_copy(y_sb, y_ps)
            nc.sync.dma_start(
                ybounce[b * T + rb * 128: b * T + (rb + 1) * 128, :], y_sb)

    if single_core:
        nc.sync.dma_start(io["Y"][:, :], ybounce)
    else:
        nc.gpsimd.collective_compute(
            "ReduceScatter", ALU.add, replica_groups=[list(range(N_CORES))],
            ins=[ybounce.opt()], outs=[yrs.opt()])
        nc.sync.dma_start(io["Y"][:, :], yrs)

    for p in (sb3, sb2, sb1, psum, dpool, cpool):
        p.release()


_CACHE = {}


def _build(single_core=False):
    key = "nc_sim" if single_core else "nc"
    if key in _CACHE:
        return _CACHE[key]
    nc = bacc.Bacc("TRN2", target_bir_lowering=False, debug=False,
                   num_devices=1 if single_core else N_CORES,
                   enable_asserts=False)
    io = {
        "AX": nc.dram_tensor("AX", [2 * B * T, D_MODEL], F32,
                             kind="ExternalInput"),
        "WB32": nc.dram_tensor("WB32", [_blob_size(WSPEC32)], F32,
                               kind="ExternalInput"),
        "WB16": nc.dram_tensor("WB16", [_blob_size(WSPEC16)], BF16,
                               kind="ExternalInput"),
        "Y": nc.dram_tensor(
            "Y", [B * T if single_core else B * T // N_CORES, D_MODEL], F32,
            kind="ExternalOutput"),
    }
    with tile.TileContext(nc) as tc:
        _emit(tc, io, single_core=single_core)
    nc.compile()
    _CACHE[key] = nc
    return nc


def _prep_in_maps(inputs):
    A = np.asarray(inputs["A"], np.float32)
    X = np.asarray(inputs["X"], np.float32)
    Wq_w = np.asarray(inputs["Wq_w"], np.float32)
    Wq_b = np.asarray(inputs["Wq_b"], np.float32)
    Wk_w = np.asarray(inputs["Wk_w"], np.float32)
    Wk_b = np.asarray(inputs["Wk_b"], np.float32)
    wedge_A = np.asarray(inputs["wedge_A"], np.float32)
    wb = np.asarray(inputs["wedge_bias"], np.float32)
    sink = np.asarray(inputs["sink_scalars"], np.float32).reshape(H_TOT)
    v_nulls = np.asarray(inputs["v_nulls"], np.float32).reshape(H_TOT, DH)
    fc_w = np.asarray(inputs["fc_w"], np.float32)
    fc_b = np.asarray(inputs["fc_b"], np.float32)
    proj_w = np.asarray(inputs["proj_w"], np.float32)
    proj_b = np.asarray(inputs["proj_b"], np.float32)
    WO = np.asarray(inputs["WO"], np.float32)
    WO_b = np.asarray(inputs["WO_b"], np.float32)

    AX = np.concatenate([A.reshape(B * T, D_MODEL),
                         X.reshape(B * T, D_MODEL)], axis=0)
    AX = np.ascontiguousarray(AX)

    skew = wedge_A - wedge_A.T
    inv_freq = 1.0 / (10000.0 ** (np.arange(0, DH, 2, dtype=np.float32) / DH))
    freqs = np.arange(T, dtype=np.float32)[:, None] * inv_freq[None, :]
    cosT = np.cos(freqs).T.astype(np.float32)
    sinT = np.sin(freqs).T.astype(np.float32)
    C2 = np.concatenate([cosT, sinT], axis=0)             # [64, T]
    S2 = np.concatenate([sinT, cosT], axis=0)
    FA = np.concatenate([fc_w[:, PERM].T, fc_b[None, :]], axis=0)
    PT = (proj_w / MLP_SCALE).T.astype(np.float32)        # [256, 64]
    PTP = np.concatenate([PT[0:128], PT[128:256]], axis=1)
    PB = proj_b[:, None]
    WOB8 = (WO_b.mean(axis=0) / N_CORES)[None, :]
    eye = np.eye(DH, dtype=np.float32)

    in_maps = []
    for c in range(N_CORES):
        h0 = c * HPC
        br = h0 // N_HEAD
        s0 = h0 % N_HEAD
        rq = np.concatenate([(h0 + h) * DH + PERM for h in range(HPC)])
        rk = np.concatenate([(s0 + h) * DH + PERM for h in range(HPC)])
        WqT = Wq_w[rq].T                                  # [768, 384]
        QB = Wq_b[rq].reshape(HPC, DH).T
        WkT = Wk_w[rk].T
        KB = Wk_b[rk].reshape(HPC, DH).T
        wedges = []
        for h in range(HPC):
            g = h0 + h
            S_h = skew + np.diag(wb[g])
            wedges.append(((eye + S_h.T)[PERM][:, PERM]).T)
        WEDGE = np.concatenate(wedges, axis=1)
        es = np.exp(sink[h0:h0 + HPC]).astype(np.float32)
        ESINK = np.broadcast_to(es[None, :], (128, HPC))
        VNS = (v_nulls[h0:h0 + HPC] * es[:, None]).reshape(1, HPC * DH)
        WOr = WO[br, s0 * DH:(s0 + HPC) * DH, :] / float(N_BR)
        vals = {
            "WkT": WkT, "WEDGE": WEDGE, "C2": C2, "S2": S2, "ESINK": ESINK,
            "VNS": VNS, "FA": FA, "PTP": PTP, "PB": PB, "WOr": WOr,
            "WOB8": WOB8, "QB": QB, "KB": KB,
        }
        wb32 = np.concatenate(
            [np.asarray(vals[n], np.float32).ravel() for n, _ in WSPEC32])
        wb16 = np.asarray(WqT, ml_dtypes.bfloat16).ravel()
        in_maps.append({"AX": AX, "WB32": np.ascontiguousarray(wb32),
                        "WB16": np.ascontiguousarray(wb16)})
    return in_maps


def run(inputs, **kwargs):
    nc = _build()
    in_maps = _prep_in_maps(inputs)
    res = run_bass_kernel_spmd(nc, in_maps, core_ids=list(range(N_CORES)),
                               **kwargs)
    parts = [res.results[c]["Y"] for c in range(N_CORES)]
    y = np.concatenate(parts, axis=0).reshape(B, T, D_MODEL)
    return y.astype(np.float32), res


def kernel(**inputs) -> np.ndarray:
    y, _ = run(inputs)
    return y


# revision 25
# speedup vs baseline: 2.3118x; 2.3118x over previous
"""Sparse-attention Bass/Tile kernel for nn_Attention_53558242181469.

SPMD over 8 NeuronCores: the 48 heads (4 branches x 12 sub-heads) are split
6-per-core (each core owns 6 contiguous sub-heads of one branch), so Wq/Wk/WO
are row-sharded too.  Per-core, everything runs as one Bass/Tile program:

  * A^T/X^T materialized straight from DRAM via xbar DMA-transposes (bf16);
    Q/K projections run in bf16 on the PE with fp32 PSUM accumulation.
    q's rmsnorm is folded into the per-row exp scale (1/sqrt(ssq+64*eps)
    absorbs both rsqrt(mean+eps) and dh^-0.5) and computed off the exact
    fp32 PSUM values.
  * BiasedWedge as a single 64x64 matmul (I + S^T, host-permuted); RoPE as
    4 DVE muls + 2 GpSimd add/subs per tensor (head dims pre-permuted to
    even/odd halves via the weight rows), emitting bf16 q_r/k_r.
  * scores = qr^T @ kr per 128-row block (only lower-triangular column
    blocks are computed); exp on the Scalar engine with accum_out giving
    the softmax denominator for free; the sink is exp(sink) host-folded.
  * top-12 via DVE max8/match_replace: top-8 of e, knock out, top-8 again;
    the 12th-largest value becomes a per-row threshold and
    masked = e * (e >= t12) * (1/(13*denom)) in two fused DVE ops.
  * masked (bf16) is transposed per 128x128 block by the DMA xbar (no PE,
    no PSUM evacuation copies) and contracted with the vanilla keys (bf16);
    the +kv/13 term is added exactly in fp32 afterward.
  * V_net MLP phase-batched per (b,h) so the Scalar engine's activation
    table is reloaded O(1) times per head instead of per row block
    (Exp/Sqrt/Silu live in different act-tables; Square/Copy/Identity are
    in every table): row blocks run matmul+Square phases inline, then one
    [1,1024] Sqrt, then one [128,2048] Silu for all 8 row blocks.
    rmsnorm-over-256 via ones-matmul + rank-1 broadcast.
  * per-branch W_O per row block (ctx stacked 6 heads -> 384 rows), WO_b
    mean/8 via a rank-1 matmul; ReduceScatter over the 8 cores sums the
    branches; each core emits its 256-row slice of the (2048,768) output.

Inputs are packed into 3 device tensors (AX16 bf16 activations, WB32/WB16
weight blobs) to minimize per-dispatch argument overhead.
"""

import numpy as np
import ml_dtypes

import concourse.mybir as mybir
import concourse.tile as tile
from concourse import bacc
from concourse.bass_utils import run_bass_kernel_spmd
from concourse.masks import make_causal_mask, make_identity

F32 = mybir.dt.float32
BF16 = mybir.dt.bfloat16
AF = mybir.ActivationFunctionType
ALU = mybir.AluOpType

D_MODEL, N_HEAD, N_BR = 768, 12, 4
DH = 64
H_TOT = 48
K_RETR = 12
MLP_SCALE = float(np.pi / np.sqrt(3.0))
N_CORES, HPC = 8, 6
B, T = 2, 1024
NRB = T // 128
EPS = float(np.finfo(np.float32).eps)
PERM = np.concatenate([np.arange(0, DH, 2), np.arange(1, DH, 2)])
NEG = -1.0e30

WSPEC32 = [
    ("C2", (DH, T)),
    ("S2", (DH, T)),
    ("ESINK", (128, HPC)),
    ("VNS", (1, HPC * DH)),
    ("PB", (DH, 1)),
    ("WOB8", (1, D_MODEL)),
    ("QB", (DH, HPC)),
    ("KB", (DH, HPC)),
]
WSPEC16 = [
    ("WqT16", (D_MODEL, HPC * DH)),
    ("WkT16", (D_MODEL, HPC * DH)),
    ("WEDGE", (DH, HPC * DH)),
    ("FA", (DH + 1, 256)),
    ("PTP", (128, 128)),
    ("WOr", (HPC * DH, D_MODEL)),
]


def _blob_views(handle, spec):
    views, off = {}, 0
    for name, (r, c) in spec:
        n = r * c
        views[name] = handle[off:off + n].rearrange("(r c) -> r c", c=c)
        off += n
    return views


def _blob_size(spec):
    return sum(r * c for _, (r, c) in spec)


def _emit(tc, io, single_core=False):
    nc = tc.nc

    cpool = tc.alloc_tile_pool(name="const", bufs=1)
    dpool = tc.alloc_tile_pool(name="dram", bufs=1, space="DRAM")
    psum = tc.alloc_tile_pool(name="psum", bufs=2, space="PSUM")
    sb1 = tc.alloc_tile_pool(name="sb1", bufs=1)
    sb2 = tc.alloc_tile_pool(name="sb2", bufs=2)
    sb4 = tc.alloc_tile_pool(name="sb4", bufs=4)

    w32 = _blob_views(io["WB32"], WSPEC32)
    w16 = _blob_views(io["WB16"], WSPEC16)
    AX16 = io["AX16"]

    # ---------------- constants -> SBUF ----------------
    wqt = [cpool.tile([128, HPC * DH], BF16, name=f"wqt{ci}") for ci in range(6)]
    wkt = [cpool.tile([128, HPC * DH], BF16, name=f"wkt{ci}") for ci in range(6)]
    for ci in range(6):
        nc.sync.dma_start(wqt[ci], w16["WqT16"][ci * 128:(ci + 1) * 128, :])
        nc.sync.dma_start(wkt[ci], w16["WkT16"][ci * 128:(ci + 1) * 128, :])
    qb_t = cpool.tile([DH, HPC], F32, name="qb_t")
    kb_t = cpool.tile([DH, HPC], F32, name="kb_t")
    nc.sync.dma_start(qb_t, w32["QB"])
    nc.sync.dma_start(kb_t, w32["KB"])
    wedge_t = cpool.tile([DH, HPC * DH], BF16, name="wedge_t")
    nc.sync.dma_start(wedge_t, w16["WEDGE"])
    c2_t = cpool.tile([DH, T], F32, name="c2_t")
    s2_t = cpool.tile([DH, T], F32, name="s2_t")
    nc.sync.dma_start(c2_t, w32["C2"])
    nc.sync.dma_start(s2_t, w32["S2"])
    esink_t = cpool.tile([128, HPC], F32, name="esink_t")
    nc.sync.dma_start(esink_t, w32["ESINK"])
    vns_t = cpool.tile([1, HPC * DH], F32, name="vns_t")
    nc.sync.dma_start(vns_t, w32["VNS"])
    fa_t = cpool.tile([DH + 1, 256], BF16, name="fa_t")
    nc.sync.dma_start(fa_t, w16["FA"])
    ptp_t = cpool.tile([128, 128], BF16, name="ptp_t")
    nc.sync.dma_start(ptp_t, w16["PTP"])
    pb_t = cpool.tile([DH, 1], F32, name="pb_t")
    nc.sync.dma_start(pb_t, w32["PB"])
    wo_t = [cpool.tile([128, D_MODEL], BF16, name=f"wo{ci}") for ci in range(3)]
    for ci in range(3):
        nc.sync.dma_start(wo_t[ci], w16["WOr"][ci * 128:(ci + 1) * 128, :])
    wob8_t = cpool.tile([1, D_MODEL], F32, name="wob8_t")
    nc.sync.dma_start(wob8_t, w32["WOB8"])

    ident = cpool.tile([128, 128], F32, name="ident")
    make_identity(nc, ident)
    cmask = cpool.tile([128, 128], F32, name="cmask")
    make_causal_mask(nc, cmask, mask_val=NEG)
    ones_row = cpool.tile([1, 128], F32, name="ones_row")
    nc.gpsimd.memset(ones_row, 1.0)
    ones_col = cpool.tile([128, 1], F32, name="ones_col")
    nc.gpsimd.memset(ones_col, 1.0)
    ones_col16 = cpool.tile([128, 1], BF16, name="ones_col16")
    nc.gpsimd.memset(ones_col16, 1.0)
    c64eps = cpool.tile([128, 1], F32, name="c64eps")
    nc.gpsimd.memset(c64eps, float(DH) * EPS)
    ceps = cpool.tile([1, 1], F32, name="ceps")
    nc.gpsimd.memset(ceps, EPS)

    ybounce = dpool.tile([B * T, D_MODEL], F32, name="ybounce")
    yrs = dpool.tile([B * T // N_CORES, D_MODEL], F32, name="yrs")

    # ---------------- main program ----------------
    for b in range(B):
        # A^T / X^T (bf16) straight from DRAM via xbar DMA transpose
        at = [sb1.tile([128, T], BF16, name=f"at{ci}", tag=f"at{ci}")
              for ci in range(6)]
        xt = [sb1.tile([128, T], BF16, name=f"xt{ci}", tag=f"xt{ci}")
              for ci in range(6)]
        for base, dst in ((b * T, at), (B * T + b * T, xt)):
            for ci in range(6):
                nc.sync.dma_start_transpose(
                    dst[ci], AX16[base:base + T, ci * 128:(ci + 1) * 128])

        ctx_tiles = [sb1.tile([128, 3 * 128], BF16, name=f"ctx{rb}", tag=f"ctx{rb}")
                     for rb in range(NRB)]

        for h in range(HPC):
            hs = slice(h * DH, (h + 1) * DH)
            # ---- Q projection ----
            qp = psum.tile([DH, T], F32, name="qp", tag="mm")
            for nh in range(2):
                ns = slice(nh * 512, (nh + 1) * 512)
                for ci in range(6):
                    nc.tensor.matmul(qp[:, ns], wqt[ci][:, hs], at[ci][:, ns],
                                     start=(ci == 0), stop=(ci == 5))
            q_sb = sb1.tile([DH, T], BF16, name="q_sb", tag="q_sb")
            nc.scalar.activation(q_sb, qp, AF.Identity, bias=qb_t[:, h:h + 1])
            sq = sb1.tile([DH, T], F32, name="sq", tag="sq")
            nc.scalar.activation(sq, qp, AF.Square, bias=qb_t[:, h:h + 1])
            ssq_ps = psum.tile([128, NRB], F32, name="ssq_ps", tag="sm")
            for rb in range(NRB):
                nc.tensor.matmul(ssq_ps[:, rb:rb + 1],
                                 sq[:, rb * 128:(rb + 1) * 128],
                                 ones_col[0:DH, :], start=True, stop=True)
            r8 = sb2.tile([128, NRB], F32, name="r8", tag="r8")
            nc.scalar.activation(r8, ssq_ps, AF.Sqrt, bias=c64eps)
            s8 = sb2.tile([128, NRB], F32, name="s8", tag="s8")
            nc.vector.reciprocal(s8, r8)

            # ---- wedge + rope q -> bf16 qr ----
            qr = sb2.tile([DH, T], BF16, name="qr", tag="qr")
            wp = psum.tile([DH, T], F32, name="wp", tag="mm")
            for nh in range(2):
                ns = slice(nh * 512, (nh + 1) * 512)
                nc.tensor.matmul(wp[:, ns], wedge_t[:, hs], q_sb[:, ns],
                                 start=True, stop=True)
            wph = sb2.tile([32, T], F32, name="wph", tag="wph")
            nc.scalar.copy(wph, wp[32:64, :])
            qa = sb2.tile([32, T], F32, name="qa", tag="ropetmp")
            qb2 = sb2.tile([32, T], F32, name="qb2", tag="ropetmp")
            nc.vector.tensor_mul(qa, wp[0:32, :], c2_t[0:32, :])
            nc.gpsimd.tensor_mul(qb2, wph, s2_t[0:32, :])
            nc.gpsimd.tensor_sub(qr[0:32, :], qa, qb2)
            qc = sb2.tile([32, T], F32, name="qc", tag="ropetmp")
            qd = sb2.tile([32, T], F32, name="qd", tag="ropetmp")
            nc.vector.tensor_mul(qc, wp[0:32, :], s2_t[0:32, :])
            nc.gpsimd.tensor_mul(qd, wph, c2_t[0:32, :])
            nc.gpsimd.tensor_add(qr[32:64, :], qc, qd)

            # ---- K projection (bf16 inputs, fp32 accum) ----
            kp = psum.tile([DH, T], F32, name="kp", tag="mm")
            for nh in range(2):
                ns = slice(nh * 512, (nh + 1) * 512)
                for ci in range(6):
                    nc.tensor.matmul(kp[:, ns], wkt[ci][:, hs], xt[ci][:, ns],
                                     start=(ci == 0), stop=(ci == 5))
            kv_sb = sb2.tile([DH, T], F32, name="kv_sb", tag="kv_sb")
            nc.scalar.activation(kv_sb, kp, AF.Identity, bias=kb_t[:, h:h + 1])
            kv16 = sb2.tile([DH, T], BF16, name="kv16", tag="kv16")
            nc.scalar.activation(kv16, kp, AF.Identity, bias=kb_t[:, h:h + 1])
            kv13 = sb1.tile([DH, T], F32, name="kv13", tag="kv13")
            nc.vector.tensor_scalar_mul(kv13, kv_sb, 1.0 / (K_RETR + 1.0))
            kr = sb2.tile([DH, T], BF16, name="kr", tag="kr")
            wpk = psum.tile([DH, T], F32, name="wpk", tag="mm")
            for nh in range(2):
                ns = slice(nh * 512, (nh + 1) * 512)
                nc.tensor.matmul(wpk[:, ns], wedge_t[:, hs], kv16[:, ns],
                                 start=True, stop=True)
            wpkh = sb2.tile([32, T], F32, name="wpkh", tag="wph")
            nc.scalar.copy(wpkh, wpk[32:64, :])
            ka = sb2.tile([32, T], F32, name="ka", tag="ropetmp")
            kb2 = sb2.tile([32, T], F32, name="kb2", tag="ropetmp")
            nc.vector.tensor_mul(ka, wpk[0:32, :], c2_t[0:32, :])
            nc.gpsimd.tensor_mul(kb2, wpkh, s2_t[0:32, :])
            nc.gpsimd.tensor_sub(kr[0:32, :], ka, kb2)
            kc = sb2.tile([32, T], F32, name="kc", tag="ropetmp")
            kd = sb2.tile([32, T], F32, name="kd", tag="ropetmp")
            nc.vector.tensor_mul(kc, wpk[0:32, :], s2_t[0:32, :])
            nc.gpsimd.tensor_mul(kd, wpkh, c2_t[0:32, :])
            nc.gpsimd.tensor_add(kr[32:64, :], kc, kd)

            # ---- vanilla keys, row layout (bf16) via DMA transpose ----
            kvrow = sb1.tile([128, NRB * DH], BF16, name="kvrow", tag="kvrow")
            for j in range(NRB):
                deng = nc.sync if j % 2 == 0 else nc.scalar
                deng.dma_start_transpose(
                    kvrow[:, j * DH:(j + 1) * DH],
                    kv16[:, j * 128:(j + 1) * 128])

            marker_sb = sb1.tile([DH + 1, T], BF16, name="marker_sb", tag="marker")
            nc.gpsimd.memset(marker_sb[DH:DH + 1, :], 1.0)
            msrow = sb1.tile([1, T], F32, name="msrow", tag="msrow")
            hp_all = sb1.tile([128, NRB * 256], F32, name="hp_all", tag="hp_all")
            rr_all = sb2.tile([1, NRB * 128], F32, name="rr_all", tag="rr_all")

            # ---- phase A: per row block through hp and mss ----
            for rb in range(NRB):
                W = 128 * (rb + 1)
                ds = slice(rb * 128, W)
                sc_ps = psum.tile([128, T], F32, name="sc_ps", tag="mm")
                for n0 in range(0, W, 512):
                    nw = min(512, W - n0)
                    nc.tensor.matmul(sc_ps[:, n0:n0 + nw], qr[:, ds],
                                     kr[:, n0:n0 + nw], start=True, stop=True)
                nc.vector.tensor_add(sc_ps[:, ds], sc_ps[:, ds], cmask)
                e_t = sb2.tile([128, T], F32, name="e_t", tag="e_t")
                acc = sb2.tile([128, 1], F32, name="acc", tag="acc")
                nc.scalar.activation(e_t[:, 0:W], sc_ps[:, 0:W], AF.Exp,
                                     scale=s8[:, rb:rb + 1], accum_out=acc)
                denom = sb2.tile([128, 1], F32, name="denom", tag="denom")
                nc.vector.tensor_scalar(denom, acc, esink_t[:, h:h + 1], None,
                                        op0=ALU.add)
                recip = sb2.tile([128, 1], F32, name="recip", tag="recip")
                nc.vector.reciprocal(recip, denom)
                recip13 = sb2.tile([128, 1], F32, name="recip13", tag="recip13")
                nc.vector.tensor_scalar_mul(recip13, recip,
                                            1.0 / (K_RETR + 1.0))
                # probs_sink row for the sink rank-1 later
                srow_ps = psum.tile([1, 128], F32, name="srow_ps", tag="sm")
                nc.tensor.matmul(srow_ps, recip, ident, start=True, stop=True)
                nc.vector.tensor_copy(rr_all[:, ds.start:ds.start + 128], srow_ps)
                # top-12 threshold
                m8a = sb2.tile([128, 8], F32, name="m8a", tag="m8a")
                nc.vector.max(out=m8a, in_=e_t[:, 0:W])
                work = sb2.tile([128, T], F32, name="work", tag="work")
                nc.vector.match_replace(out=work[:, 0:W], in_to_replace=m8a,
                                        in_values=e_t[:, 0:W], imm_value=0.0)
                m8b = sb2.tile([128, 8], F32, name="m8b", tag="m8b")
                nc.vector.max(out=m8b, in_=work[:, 0:W])
                nc.vector.tensor_scalar(work[:, 0:W], e_t[:, 0:W],
                                        m8b[:, 3:4], recip13,
                                        op0=ALU.is_ge, op1=ALU.mult)
                masked = sb2.tile([128, T], BF16, name="masked", tag="masked")
                nc.vector.tensor_mul(masked[:, 0:W], work[:, 0:W], e_t[:, 0:W])
                # marker^T = masked @ kv (bf16 matmul) + kv/13 (fp32)
                mk_ps = psum.tile([DH, 128], F32, name="mk_ps", tag="mk")
                for j in range(rb + 1):
                    mT = sb4.tile([128, 128], BF16, name="mT", tag="mT")
                    deng = nc.sync if j % 2 == 0 else nc.scalar
                    deng.dma_start_transpose(
                        mT, masked[:, j * 128:(j + 1) * 128])
                    nc.tensor.matmul(mk_ps, kvrow[:, j * DH:(j + 1) * DH], mT,
                                     start=(j == 0), stop=(j == rb))
                nc.vector.tensor_add(marker_sb[0:DH, ds], mk_ps, kv13[:, ds])
                # MLP front half: h1, hp = h1^2*(1+0.75*h1), mss = sum(hp^2)
                h1_ps = psum.tile([128, 256], F32, name="h1_ps", tag="sm")
                nc.tensor.matmul(h1_ps[:, 0:128], fa_t[:, 0:128],
                                 marker_sb[:, ds], start=True, stop=True)
                nc.tensor.matmul(h1_ps[:, 128:256], fa_t[:, 128:256],
                                 marker_sb[:, ds], start=True, stop=True)
                s1 = sb2.tile([128, 256], F32, name="s1", tag="s1")
                nc.vector.tensor_scalar(s1, h1_ps, 0.75, 1.0,
                                        op0=ALU.mult, op1=ALU.add)
                sqm = sb2.tile([128, 256], F32, name="sqm", tag="sqm")
                nc.scalar.activation(sqm, h1_ps, AF.Square)
                hps = hp_all[:, rb * 256:(rb + 1) * 256]
                nc.vector.tensor_mul(hps, sqm, s1)
                sq2 = sb2.tile([128, 256], BF16, name="sq2", tag="sq2")
                nc.scalar.activation(sq2, hps, AF.Square)
                mss_ps = psum.tile([1, 128], F32, name="mss_ps", tag="sm")
                nc.tensor.matmul(mss_ps, ones_col16, sq2[:, 0:128],
                                 start=True, stop=False)
                nc.tensor.matmul(mss_ps, ones_col16, sq2[:, 128:256],
                                 start=False, stop=True)
                nc.scalar.copy(msrow[:, rb * 128:(rb + 1) * 128], mss_ps)

            # ---- phase B: one Sqrt for all 8 row blocks ----
            rms_all = sb1.tile([1, T], F32, name="rms_all", tag="rms_all")
            nc.scalar.activation(rms_all, msrow, AF.Sqrt, bias=ceps,
                                 scale=1.0 / 256.0)
            rcol_ps = psum.tile([128, NRB], F32, name="rcol_ps", tag="sm")
            for rb in range(NRB):
                nc.tensor.matmul(rcol_ps[:, rb:rb + 1],
                                 rms_all[:, rb * 128:(rb + 1) * 128],
                                 ones_row[0:1, 0:1], start=True, stop=True)
            invcol = sb2.tile([128, NRB], F32, name="invcol", tag="invcol")
            nc.vector.reciprocal(invcol, rcol_ps)

            # ---- phase C: hn = hp * inv (rank-1 broadcast per row block) ----
            for rb in range(NRB):
                invrow_ps = psum.tile([1, 128], F32, name="invrow_ps", tag="sm")
                nc.tensor.matmul(invrow_ps, invcol[:, rb:rb + 1], ident,
                                 start=True, stop=True)
                invrow = sb2.tile([1, 128], F32, name="invrow", tag="invrow")
                nc.vector.tensor_copy(invrow, invrow_ps)
                invbc_ps = psum.tile([128, 128], F32, name="invbc_ps", tag="sm")
                nc.tensor.matmul(invbc_ps, ones_row, invrow,
                                 start=True, stop=True)
                hp3 = hp_all[:, rb * 256:(rb + 1) * 256].rearrange(
                    "p (two x) -> p two x", two=2)
                inv_b = invbc_ps.rearrange("p (one x) -> p one x", one=1)
                nc.vector.tensor_tensor(hp3, hp3,
                                        inv_b.to_broadcast([128, 2, 128]),
                                        op=ALU.mult)

            # ---- phase D: one Silu for all 8 row blocks ----
            hf_all = sb1.tile([128, NRB * 256], BF16, name="hf_all", tag="hf_all")
            nc.scalar.activation(hf_all, hp_all, AF.Silu, scale=MLP_SCALE)

            # ---- phase E: proj + sink + ctx ----
            for rb in range(NRB):
                fs = slice(rb * 256, rb * 256 + 128)
                fs2 = slice(rb * 256 + 128, (rb + 1) * 256)
                ot_ps = psum.tile([DH, 128], F32, name="ot_ps", tag="mk")
                nc.tensor.matmul(ot_ps, ptp_t[:, 0:DH], hf_all[:, fs],
                                 start=True, stop=False)
                nc.tensor.matmul(ot_ps, ptp_t[:, DH:128], hf_all[:, fs2],
                                 start=False, stop=False)
                nc.tensor.matmul(ot_ps, vns_t[0:1, hs],
                                 rr_all[:, rb * 128:(rb + 1) * 128],
                                 start=False, stop=True)
                nc.scalar.activation(
                    ctx_tiles[rb][DH * (h % 2):DH * (h % 2) + DH,
                                  128 * (h // 2):128 * (h // 2) + 128],
                    ot_ps, AF.Identity, bias=pb_t)

        # ---- output projection + bias per row block ----
        for rb in range(NRB):
            y_ps = psum.tile([128, D_MODEL], F32, name="y_ps", tag="mm")
            for n0, nw in ((0, 512), (512, 256)):
                for ci in range(3):
                    nc.tensor.matmul(y_ps[:, n0:n0 + nw],
                                     ctx_tiles[rb][:, ci * 128:(ci + 1) * 128],
                                     wo_t[ci][:, n0:n0 + nw],
                                     start=(ci == 0), stop=False)
                nc.tensor.matmul(y_ps[:, n0:n0 + nw], ones_row,
                                 wob8_t[0:1, n0:n0 + nw], start=False, stop=True)
            y_sb = sb2.tile([128, D_MODEL], F32, name="y_sb", tag="y_sb")
            nc.vector.tensor_copy(y_sb, y_ps)
            nc.sync.dma_start(
                ybounce[b * T + rb * 128: b * T + (rb + 1) * 128, :], y_sb)

    if single_core:
        nc.sync.dma_start(io["Y"][:, :], ybounce)
    else:
        nc.gpsimd.collective_compute(
            "ReduceScatter", ALU.add, replica_groups=[list(range(N_CORES))],
            ins=[ybounce.opt()], outs=[yrs.opt()])
        nc.sync.dma_start(io["Y"][:, :], yrs)

    for p in (sb4, sb2, sb1, psum, dpool, cpool):
        p.release()


_CACHE = {}


def _build(single_core=False):
    key = "nc_sim" if single_core else "nc"
    if key in _CACHE:
        return _CACHE[key]
    nc = bacc.Bacc("TRN2", target_bir_lowering=False, debug=False,
                   num_devices=1 if single_core else N_CORES,
                   enable_asserts=False)
    io = {
        "AX16": nc.dram_tensor("AX16", [2 * B * T, D_MODEL], BF16,
                               kind="ExternalInput"),
        "WB32": nc.dram_tensor("WB32", [_blob_size(WSPEC32)], F32,
                               kind="ExternalInput"),
        "WB16": nc.dram_tensor("WB16", [_blob_size(WSPEC16)], BF16,
                               kind="ExternalInput"),
        "Y": nc.dram_tensor(
            "Y", [B * T if single_core else B * T // N_CORES, D_MODEL], F32,
            kind="ExternalOutput"),
    }
    with tile.TileContext(nc) as tc:
        _emit(tc, io, single_core=single_core)
    nc.compile()
    _CACHE[key] = nc
    return nc


def _prep_in_maps(inputs):
    A = np.asarray(inputs["A"], np.float32)
    X = np.asarray(inputs["X"], np.float32)
    Wq_w = np.asarray(inputs["Wq_w"], np.float32)
    Wq_b = np.asarray(inputs["Wq_b"], np.float32)
    Wk_w = np.asarray(inputs["Wk_w"], np.float32)
    Wk_b = np.asarray(inputs["Wk_b"], np.float32)
    wedge_A = np.asarray(inputs["wedge_A"], np.float32)
    wb = np.asarray(inputs["wedge_bias"], np.float32)
    sink = np.asarray(inputs["sink_scalars"], np.float32).reshape(H_TOT)
    v_nulls = np.asarray(inputs["v_nulls"], np.float32).reshape(H_TOT, DH)
    fc_w = np.asarray(inputs["fc_w"], np.float32)
    fc_b = np.asarray(inputs["fc_b"], np.float32)
    proj_w = np.asarray(inputs["proj_w"], np.float32)
    proj_b = np.asarray(inputs["proj_b"], np.float32)
    WO = np.asarray(inputs["WO"], np.float32)
    WO_b = np.asarray(inputs["WO_b"], np.float32)

    AX = np.concatenate([A.reshape(B * T, D_MODEL),
                         X.reshape(B * T, D_MODEL)], axis=0)
    AX16 = np.ascontiguousarray(AX.astype(ml_dtypes.bfloat16))

    skew = wedge_A - wedge_A.T
    inv_freq = 1.0 / (10000.0 ** (np.arange(0, DH, 2, dtype=np.float32) / DH))
    freqs = np.arange(T, dtype=np.float32)[:, None] * inv_freq[None, :]
    cosT = np.cos(freqs).T.astype(np.float32)
    sinT = np.sin(freqs).T.astype(np.float32)
    C2 = np.concatenate([cosT, sinT], axis=0)
    S2 = np.concatenate([sinT, cosT], axis=0)
    FA = np.concatenate([fc_w[:, PERM].T, fc_b[None, :]], axis=0)
    PT = (proj_w / MLP_SCALE).T.astype(np.float32)
    PTP = np.concatenate([PT[0:128], PT[128:256]], axis=1)
    PB = proj_b[:, None]
    WOB8 = (WO_b.mean(axis=0) / N_CORES)[None, :]
    eye = np.eye(DH, dtype=np.float32)

    in_maps = []
    for c in range(N_CORES):
        h0 = c * HPC
        br = h0 // N_HEAD
        s0 = h0 % N_HEAD
        rq = np.concatenate([(h0 + h) * DH + PERM for h in range(HPC)])
        rk = np.concatenate([(s0 + h) * DH + PERM for h in range(HPC)])
        WqT = Wq_w[rq].T
        QB = Wq_b[rq].reshape(HPC, DH).T
        WkT = Wk_w[rk].T
        KB = Wk_b[rk].reshape(HPC, DH).T
        wedges = []
        for h in range(HPC):
            g = h0 + h
            S_h = skew + np.diag(wb[g])
            wedges.append(((eye + S_h.T)[PERM][:, PERM]).T)
        WEDGE = np.concatenate(wedges, axis=1)
        es = np.exp(sink[h0:h0 + HPC]).astype(np.float32)
        ESINK = np.broadcast_to(es[None, :], (128, HPC))
        VNS = (v_nulls[h0:h0 + HPC] * es[:, None]).reshape(1, HPC * DH)
        WOr = WO[br, s0 * DH:(s0 + HPC) * DH, :] / float(N_BR)
        vals = {
            "C2": C2, "S2": S2, "ESINK": ESINK,
            "VNS": VNS, "PB": PB,
            "WOB8": WOB8, "QB": QB, "KB": KB,
        }
        vals16 = {
            "WqT16": WqT, "WkT16": WkT, "WEDGE": WEDGE, "FA": FA,
            "PTP": PTP, "WOr": WOr,
        }
        wb32 = np.concatenate(
            [np.asarray(vals[n], np.float32).ravel() for n, _ in WSPEC32])
        wb16 = np.concatenate(
            [np.asarray(vals16[n], ml_dtypes.bfloat16).ravel()
             for n, _ in WSPEC16])
        in_maps.append({"AX16": AX16, "WB32": np.ascontiguousarray(wb32),
                        "WB16": np.ascontiguousarray(wb16)})
    return in_maps


def run(inputs, **kwargs):
    nc = _build()
    in_maps = _prep_in_maps(inputs)
    res = run_bass_kernel_spmd(nc, in_maps, core_ids=list(range(N_CORES)),
                               **kwargs)
    parts = [res.results[c]["Y"] for c in range(N_CORES)]
    y = np.concatenate(parts, axis=0).reshape(B, T, D_MODEL)
    return y.astype(np.float32), res


def kernel(**inputs) -> np.ndarray:
    y, _ = run(inputs)
    return y


# revision 36
# speedup vs baseline: 2.3400x; 1.0122x over previous
"""Sparse-attention Bass/Tile kernel for nn_Attention_53558242181469.

SPMD over 8 NeuronCores: the 48 heads (4 branches x 12 sub-heads) are split
6-per-core (each core owns 6 contiguous sub-heads of one branch), so Wq/Wk/WO
are row-sharded too.  Per-core, everything runs as one Bass/Tile program:

  * A^T/X^T materialized straight from DRAM via xbar DMA-transposes (bf16);
    Q/K projections run in bf16 on the PE with fp32 PSUM accumulation.
    q's rmsnorm is folded into the per-row exp scale (1/sqrt(ssq+64*eps)
    absorbs both rsqrt(mean+eps) and dh^-0.5) and computed off the exact
    fp32 PSUM values.
  * BiasedWedge as a single 64x64 matmul (I + S^T, host-permuted); RoPE as
    4 DVE muls + 2 GpSimd add/subs per tensor (head dims pre-permuted to
    even/odd halves via the weight rows), emitting bf16 q_r/k_r.
  * scores = qr^T @ kr per 128-row block (only lower-triangular column
    blocks are computed); exp on the Scalar engine with accum_out giving
    the softmax denominator for free; the sink is exp(sink) host-folded.
  * top-12 via DVE max8/match_replace: top-8 of e, knock out, top-8 again;
    the 12th-largest value becomes a per-row threshold and
    masked = e * (e >= t12) * (1/(13*denom)) in two fused DVE ops.
  * masked (bf16) is transposed per 128x128 block by the DMA xbar (no PE,
    no PSUM evacuation copies) and contracted with the vanilla keys (bf16);
    the +kv/13 term is added exactly in fp32 afterward.
  * V_net MLP phase-batched per (b,h) so the Scalar engine's activation
    table is reloaded O(1) times per head instead of per row block
    (Exp/Sqrt/Silu live in different act-tables; Square/Copy/Identity are
    in every table): row blocks run matmul+Square phases inline, then one
    [1,1024] Sqrt, then one [128,2048] Silu for all 8 row blocks.
    rmsnorm-over-256 via ones-matmul + rank-1 broadcast.
  * per-branch W_O per row block (ctx stacked 6 heads -> 384 rows), WO_b
    mean/8 via a rank-1 matmul; ReduceScatter over the 8 cores sums the
    branches; each core emits its 256-row slice of the (2048,768) output.

Inputs are packed into 3 device tensors (AX16 bf16 activations, WB32/WB16
weight blobs) to minimize per-dispatch argument overhead.
"""

import numpy as np
import ml_dtypes

import concourse.mybir as mybir
import concourse.tile as tile
from concourse import bacc
from concourse.bass_utils import run_bass_kernel_spmd
from concourse.masks import make_causal_mask, make_identity

F32 = mybir.dt.float32
BF16 = mybir.dt.bfloat16
AF = mybir.ActivationFunctionType
ALU = mybir.AluOpType

D_MODEL, N_HEAD, N_BR = 768, 12, 4
DH = 64
H_TOT = 48
K_RETR = 12
MLP_SCALE = float(np.pi / np.sqrt(3.0))
N_CORES, HPC = 8, 6
B, T = 2, 1024
NRB = T // 128
EPS = float(np.finfo(np.float32).eps)
PERM = np.concatenate([np.arange(0, DH, 2), np.arange(1, DH, 2)])
NEG = -1.0e30

WSPEC32 = [
    ("C2", (DH, T)),
    ("S2", (DH, T)),
    ("ESINK", (128, HPC)),
    ("VNS", (1, HPC * DH)),
    ("PB", (DH, 1)),
    ("WOB8", (1, D_MODEL)),
    ("QB", (DH, HPC)),
    ("KB", (DH, HPC)),
]
WSPEC16 = [
    ("WqT16", (D_MODEL, HPC * DH)),
    ("WkT16", (D_MODEL, HPC * DH)),
    ("WEDGE", (DH, HPC * DH)),
    ("FA", (DH + 1, 256)),
    ("PTP", (128, 128)),
    ("WOr", (HPC * DH, D_MODEL)),
]


def _blob_views(handle, spec):
    views, off = {}, 0
    for name, (r, c) in spec:
        n = r * c
        views[name] = handle[off:off + n].rearrange("(r c) -> r c", c=c)
        off += n
    return views


def _blob_size(spec):
    return sum(r * c for _, (r, c) in spec)


def _emit(tc, io, single_core=False, reps=1):
    nc = tc.nc

    cpool = tc.alloc_tile_pool(name="const", bufs=1)
    dpool = tc.alloc_tile_pool(name="dram", bufs=1, space="DRAM")
    psum = tc.alloc_tile_pool(name="psum", bufs=2, space="PSUM")
    sb1 = tc.alloc_tile_pool(name="sb1", bufs=1)
    sb2 = tc.alloc_tile_pool(name="sb2", bufs=2)
    sb3 = tc.alloc_tile_pool(name="sb3", bufs=3)
    sb4 = tc.alloc_tile_pool(name="sb4", bufs=4)

    w32 = _blob_views(io["WB32"], WSPEC32)
    w16 = _blob_views(io["WB16"], WSPEC16)
    AX16 = io["AX16"]

    # ---------------- constants -> SBUF ----------------
    wqt = [cpool.tile([128, HPC * DH], BF16, name=f"wqt{ci}") for ci in range(6)]
    wkt = [cpool.tile([128, HPC * DH], BF16, name=f"wkt{ci}") for ci in range(6)]
    for ci in range(6):
        nc.sync.dma_start(wqt[ci], w16["WqT16"][ci * 128:(ci + 1) * 128, :])
        nc.sync.dma_start(wkt[ci], w16["WkT16"][ci * 128:(ci + 1) * 128, :])
    qb_t = cpool.tile([DH, HPC], F32, name="qb_t")
    kb_t = cpool.tile([DH, HPC], F32, name="kb_t")
    nc.sync.dma_start(qb_t, w32["QB"])
    nc.sync.dma_start(kb_t, w32["KB"])
    wedge_t = cpool.tile([DH, HPC * DH], BF16, name="wedge_t")
    nc.sync.dma_start(wedge_t, w16["WEDGE"])
    c2_t = cpool.tile([DH, T], F32, name="c2_t")
    s2_t = cpool.tile([DH, T], F32, name="s2_t")
    nc.sync.dma_start(c2_t, w32["C2"])
    nc.sync.dma_start(s2_t, w32["S2"])
    esink_t = cpool.tile([128, HPC], F32, name="esink_t")
    nc.sync.dma_start(esink_t, w32["ESINK"])
    vns_t = cpool.tile([1, HPC * DH], F32, name="vns_t")
    nc.sync.dma_start(vns_t, w32["VNS"])
    fa_t = cpool.tile([DH + 1, 256], BF16, name="fa_t")
    nc.sync.dma_start(fa_t, w16["FA"])
    ptp_t = cpool.tile([128, 128], BF16, name="ptp_t")
    nc.sync.dma_start(ptp_t, w16["PTP"])
    pb_t = cpool.tile([DH, 1], F32, name="pb_t")
    nc.sync.dma_start(pb_t, w32["PB"])
    wo_t = [cpool.tile([128, D_MODEL], BF16, name=f"wo{ci}") for ci in range(3)]
    for ci in range(3):
        nc.sync.dma_start(wo_t[ci], w16["WOr"][ci * 128:(ci + 1) * 128, :])
    wob8_t = cpool.tile([1, D_MODEL], F32, name="wob8_t")
    nc.sync.dma_start(wob8_t, w32["WOB8"])

    ident = cpool.tile([128, 128], F32, name="ident")
    make_identity(nc, ident)
    identb = cpool.tile([128, 128], BF16, name="identb")
    make_identity(nc, identb)
    cmask = cpool.tile([128, 128], F32, name="cmask")
    make_causal_mask(nc, cmask, mask_val=NEG)
    ones_row = cpool.tile([1, 128], F32, name="ones_row")
    nc.gpsimd.memset(ones_row, 1.0)
    ones_col = cpool.tile([128, 1], F32, name="ones_col")
    nc.gpsimd.memset(ones_col, 1.0)
    ones_col16 = cpool.tile([128, 1], BF16, name="ones_col16")
    nc.gpsimd.memset(ones_col16, 1.0)
    c64eps = cpool.tile([128, 1], F32, name="c64eps")
    nc.gpsimd.memset(c64eps, float(DH) * EPS)
    ceps = cpool.tile([1, 1], F32, name="ceps")
    nc.gpsimd.memset(ceps, EPS)

    ybounce = dpool.tile([B * T, D_MODEL], F32, name="ybounce")
    yrs = dpool.tile([B * T // N_CORES, D_MODEL], F32, name="yrs")

    # ---------------- main program ----------------
    for b in list(range(B)) * reps:
        # A^T / X^T (bf16) straight from DRAM via xbar DMA transpose
        at = [sb1.tile([128, T], BF16, name=f"at{ci}", tag=f"at{ci}")
              for ci in range(6)]
        xt = [sb1.tile([128, T], BF16, name=f"xt{ci}", tag=f"xt{ci}")
              for ci in range(6)]
        for base, dst in ((b * T, at), (B * T + b * T, xt)):
            for rt in range(NRB):
                arow = sb4.tile([128, D_MODEL], BF16, name="arow", tag="arow")
                nc.sync.dma_start(
                    arow, AX16[base + rt * 128: base + (rt + 1) * 128, :])
                for ci in range(6):
                    tpa = psum.tile([128, 128], BF16, name="tpa", tag="sm")
                    nc.tensor.transpose(tpa, arow[:, ci * 128:(ci + 1) * 128],
                                        identb)
                    if ci % 2 == 0:
                        nc.scalar.copy(dst[ci][:, rt * 128:(rt + 1) * 128], tpa)
                    else:
                        nc.vector.tensor_copy(
                            dst[ci][:, rt * 128:(rt + 1) * 128], tpa)

        ctx_tiles = [sb1.tile([128, 3 * 128], BF16, name=f"ctx{rb}", tag=f"ctx{rb}")
                     for rb in range(NRB)]

        for h in range(HPC):
            hs = slice(h * DH, (h + 1) * DH)
            # ---- Q projection ----
            qp = psum.tile([DH, T], F32, name="qp", tag="mm")
            for nh in range(2):
                ns = slice(nh * 512, (nh + 1) * 512)
                for ci in range(6):
                    nc.tensor.matmul(qp[:, ns], wqt[ci][:, hs], at[ci][:, ns],
                                     start=(ci == 0), stop=(ci == 5))
            q_sb = sb1.tile([DH, T], BF16, name="q_sb", tag="q_sb")
            nc.scalar.activation(q_sb, qp, AF.Identity, bias=qb_t[:, h:h + 1])
            sq = sb1.tile([DH, T], F32, name="sq", tag="sq")
            nc.scalar.activation(sq, qp, AF.Square, bias=qb_t[:, h:h + 1])
            ssq_ps = psum.tile([128, NRB], F32, name="ssq_ps", tag="sm")
            for rb in range(NRB):
                nc.tensor.matmul(ssq_ps[:, rb:rb + 1],
                                 sq[:, rb * 128:(rb + 1) * 128],
                                 ones_col[0:DH, :], start=True, stop=True)
            r8 = sb2.tile([128, NRB], F32, name="r8", tag="r8")
            nc.scalar.activation(r8, ssq_ps, AF.Sqrt, bias=c64eps)
            s8 = sb2.tile([128, NRB], F32, name="s8", tag="s8")
            nc.vector.reciprocal(s8, r8)

            # ---- wedge + rope q -> bf16 qr ----
            qr = sb2.tile([DH, T], BF16, name="qr", tag="qr")
            wp = psum.tile([DH, T], F32, name="wp", tag="mm")
            for nh in range(2):
                ns = slice(nh * 512, (nh + 1) * 512)
                nc.tensor.matmul(wp[:, ns], wedge_t[:, hs], q_sb[:, ns],
                                 start=True, stop=True)
            wph = sb2.tile([32, T], F32, name="wph", tag="wph")
            nc.scalar.copy(wph, wp[32:64, :])
            qa = sb2.tile([32, T], F32, name="qa", tag="ropetmp")
            qb2 = sb2.tile([32, T], F32, name="qb2", tag="ropetmp")
            nc.vector.tensor_mul(qa, wp[0:32, :], c2_t[0:32, :])
            nc.gpsimd.tensor_mul(qb2, wph, s2_t[0:32, :])
            nc.gpsimd.tensor_sub(qr[0:32, :], qa, qb2)
            qc = sb2.tile([32, T], F32, name="qc", tag="ropetmp")
            qd = sb2.tile([32, T], F32, name="qd", tag="ropetmp")
            nc.vector.tensor_mul(qc, wp[0:32, :], s2_t[0:32, :])
            nc.gpsimd.tensor_mul(qd, wph, c2_t[0:32, :])
            nc.gpsimd.tensor_add(qr[32:64, :], qc, qd)

            # ---- K projection (bf16 inputs, fp32 accum) ----
            kp = psum.tile([DH, T], F32, name="kp", tag="mm")
            for nh in range(2):
                ns = slice(nh * 512, (nh + 1) * 512)
                for ci in range(6):
                    nc.tensor.matmul(kp[:, ns], wkt[ci][:, hs], xt[ci][:, ns],
                                     start=(ci == 0), stop=(ci == 5))
            kv_sb = sb2.tile([DH, T], F32, name="kv_sb", tag="kv_sb")
            nc.scalar.activation(kv_sb, kp, AF.Identity, bias=kb_t[:, h:h + 1])
            kv16 = sb2.tile([DH, T], BF16, name="kv16", tag="kv16")
            nc.scalar.activation(kv16, kp, AF.Identity, bias=kb_t[:, h:h + 1])
            kv13 = sb1.tile([DH, T], F32, name="kv13", tag="kv13")
            nc.vector.tensor_scalar_mul(kv13, kv_sb, 1.0 / (K_RETR + 1.0))
            kr = sb2.tile([DH, T], BF16, name="kr", tag="kr")
            wpk = psum.tile([DH, T], F32, name="wpk", tag="mm")
            for nh in range(2):
                ns = slice(nh * 512, (nh + 1) * 512)
                nc.tensor.matmul(wpk[:, ns], wedge_t[:, hs], kv16[:, ns],
                                 start=True, stop=True)
            wpkh = sb2.tile([32, T], F32, name="wpkh", tag="wph")
            nc.scalar.copy(wpkh, wpk[32:64, :])
            ka = sb2.tile([32, T], F32, name="ka", tag="ropetmp")
            kb2 = sb2.tile([32, T], F32, name="kb2", tag="ropetmp")
            nc.vector.tensor_mul(ka, wpk[0:32, :], c2_t[0:32, :])
            nc.gpsimd.tensor_mul(kb2, wpkh, s2_t[0:32, :])
            nc.gpsimd.tensor_sub(kr[0:32, :], ka, kb2)
            kc = sb2.tile([32, T], F32, name="kc", tag="ropetmp")
            kd = sb2.tile([32, T], F32, name="kd", tag="ropetmp")
            nc.vector.tensor_mul(kc, wpk[0:32, :], s2_t[0:32, :])
            nc.gpsimd.tensor_mul(kd, wpkh, c2_t[0:32, :])
            nc.gpsimd.tensor_add(kr[32:64, :], kc, kd)

            # ---- vanilla keys, row layout (bf16) via DMA transpose ----
            kvrow = sb1.tile([128, NRB * DH], BF16, name="kvrow", tag="kvrow")
            for j in range(NRB):
                tpk = psum.tile([128, DH], BF16, name="tpk", tag="sm")
                nc.tensor.transpose(tpk, kv16[:, j * 128:(j + 1) * 128],
                                    identb[0:DH, 0:DH])
                nc.scalar.copy(kvrow[:, j * DH:(j + 1) * DH], tpk)

            marker_sb = sb1.tile([DH + 1, T], BF16, name="marker_sb", tag="marker")
            nc.gpsimd.memset(marker_sb[DH:DH + 1, :], 1.0)
            msrow = sb2.tile([1, T], F32, name="msrow", tag="msrow")
            hp_all = sb2.tile([128, NRB * 256], F32, name="hp_all", tag="hp_all")
            rr_all = sb2.tile([1, NRB * 128], F32, name="rr_all", tag="rr_all")

            # ---- phase A: per row block through hp and mss ----
            for rb in range(NRB):
                W = 128 * (rb + 1)
                ds = slice(rb * 128, W)
                sc_ps = psum.tile([128, T], F32, name="sc_ps", tag="mm")
                for n0 in range(0, W, 512):
                    nw = min(512, W - n0)
                    nc.tensor.matmul(sc_ps[:, n0:n0 + nw], qr[:, ds],
                                     kr[:, n0:n0 + nw], start=True, stop=True)
                nc.vector.tensor_add(sc_ps[:, ds], sc_ps[:, ds], cmask)
                e_t = sb2.tile([128, T], F32, name="e_t", tag="e_t")
                acc = sb2.tile([128, 1], F32, name="acc", tag="acc")
                nc.scalar.activation(e_t[:, 0:W], sc_ps[:, 0:W], AF.Exp,
                                     scale=s8[:, rb:rb + 1], accum_out=acc)
                denom = sb2.tile([128, 1], F32, name="denom", tag="denom")
                nc.vector.tensor_scalar(denom, acc, esink_t[:, h:h + 1], None,
                                        op0=ALU.add)
                recip = sb2.tile([128, 1], F32, name="recip", tag="recip")
                nc.vector.reciprocal(recip, denom)
                recip13 = sb2.tile([128, 1], F32, name="recip13", tag="recip13")
                nc.vector.tensor_scalar_mul(recip13, recip,
                                            1.0 / (K_RETR + 1.0))
                # probs_sink row for the sink rank-1 later
                srow_ps = psum.tile([1, 128], F32, name="srow_ps", tag="sm")
                nc.tensor.matmul(srow_ps, recip, ident, start=True, stop=True)
                nc.scalar.copy(rr_all[:, ds.start:ds.start + 128], srow_ps)
                # top-12 threshold
                m8a = sb2.tile([128, 8], F32, name="m8a", tag="m8a")
                nc.vector.max(out=m8a, in_=e_t[:, 0:W])
                work = sb2.tile([128, T], F32, name="work", tag="work")
                nc.vector.match_replace(out=work[:, 0:W], in_to_replace=m8a,
                                        in_values=e_t[:, 0:W], imm_value=0.0)
                m8b = sb2.tile([128, 8], F32, name="m8b", tag="m8b")
                nc.vector.max(out=m8b, in_=work[:, 0:W])
                nc.vector.tensor_scalar(work[:, 0:W], e_t[:, 0:W],
                                        m8b[:, 3:4], recip13,
                                        op0=ALU.is_ge, op1=ALU.mult)
                masked = sb2.tile([128, T], BF16, name="masked", tag="masked")
                nc.vector.tensor_mul(masked[:, 0:W], work[:, 0:W], e_t[:, 0:W])
                # marker^T = masked @ kv (bf16 matmul) + kv/13 (fp32)
                mk_ps = psum.tile([DH, 128], F32, name="mk_ps", tag="mk")
                for j in range(rb + 1):
                    mT = sb4.tile([128, 128], BF16, name="mT", tag="mT")
                    tpm = psum.tile([128, 128], BF16, name="tpm", tag="sm")
                    nc.tensor.transpose(tpm, masked[:, j * 128:(j + 1) * 128],
                                        identb)
                    if j % 2 == 0:
                        nc.scalar.copy(mT, tpm)
                    else:
                        nc.vector.tensor_copy(mT, tpm)
                    nc.tensor.matmul(mk_ps, kvrow[:, j * DH:(j + 1) * DH], mT,
                                     start=(j == 0), stop=(j == rb))
                nc.vector.tensor_add(marker_sb[0:DH, ds], mk_ps, kv13[:, ds])
                # MLP front half: h1, hp = h1^2*(1+0.75*h1), mss = sum(hp^2)
                h1_ps = psum.tile([128, 256], F32, name="h1_ps", tag="sm")
                nc.tensor.matmul(h1_ps[:, 0:128], fa_t[:, 0:128],
                                 marker_sb[:, ds], start=True, stop=True)
                nc.tensor.matmul(h1_ps[:, 128:256], fa_t[:, 128:256],
                                 marker_sb[:, ds], start=True, stop=True)
                s1 = sb2.tile([128, 256], F32, name="s1", tag="s1")
                nc.vector.tensor_scalar(s1, h1_ps, 0.75, 1.0,
                                        op0=ALU.mult, op1=ALU.add)
                sqm = sb2.tile([128, 256], F32, name="sqm", tag="sqm")
                nc.scalar.activation(sqm, h1_ps, AF.Square)
                hps = hp_all[:, rb * 256:(rb + 1) * 256]
                nc.vector.tensor_mul(hps, sqm, s1)
                sq2 = sb2.tile([128, 256], BF16, name="sq2", tag="sq2")
                nc.scalar.activation(sq2, hps, AF.Square)
                mss_ps = psum.tile([1, 128], F32, name="mss_ps", tag="sm")
                nc.tensor.matmul(mss_ps, ones_col16, sq2[:, 0:128],
                                 start=True, stop=False)
                nc.tensor.matmul(mss_ps, ones_col16, sq2[:, 128:256],
                                 start=False, stop=True)
                nc.scalar.copy(msrow[:, rb * 128:(rb + 1) * 128], mss_ps)

            # ---- phase B: one Sqrt for all 8 row blocks ----
            rms_all = sb2.tile([1, T], F32, name="rms_all", tag="rms_all")
            nc.scalar.activation(rms_all, msrow, AF.Sqrt, bias=ceps,
                                 scale=1.0 / 256.0)
            rcol_ps = psum.tile([128, NRB], F32, name="rcol_ps", tag="sm")
            for rb in range(NRB):
                nc.tensor.matmul(rcol_ps[:, rb:rb + 1],
                                 rms_all[:, rb * 128:(rb + 1) * 128],
                                 ones_row[0:1, 0:1], start=True, stop=True)
            invcol = sb2.tile([128, NRB], F32, name="invcol", tag="invcol")
            nc.vector.reciprocal(invcol, rcol_ps)

            # ---- phase C: hn = hp * inv (rank-1 broadcast per row block) ----
            for rb in range(NRB):
                invrow_ps = psum.tile([1, 128], F32, name="invrow_ps", tag="sm")
                nc.tensor.matmul(invrow_ps, invcol[:, rb:rb + 1], ident,
                                 start=True, stop=True)
                invrow = sb2.tile([1, 128], F32, name="invrow", tag="invrow")
                nc.vector.tensor_copy(invrow, invrow_ps)
                invbc_ps = psum.tile([128, 128], F32, name="invbc_ps", tag="sm")
                nc.tensor.matmul(invbc_ps, ones_row, invrow,
                                 start=True, stop=True)
                hp3 = hp_all[:, rb * 256:(rb + 1) * 256].rearrange(
                    "p (two x) -> p two x", two=2)
                inv_b = invbc_ps.rearrange("p (one x) -> p one x", one=1)
                nc.vector.tensor_tensor(hp3, hp3,
                                        inv_b.to_broadcast([128, 2, 128]),
                                        op=ALU.mult)

            # ---- phase D: one Silu for all 8 row blocks ----
            hf_all = sb2.tile([128, NRB * 256], BF16, name="hf_all", tag="hf_all")
            nc.scalar.activation(hf_all, hp_all, AF.Silu, scale=MLP_SCALE)

            # ---- phase E: proj + sink + ctx ----
            for rb in range(NRB):
                fs = slice(rb * 256, rb * 256 + 128)
                fs2 = slice(rb * 256 + 128, (rb + 1) * 256)
                ot_ps = psum.tile([DH, 128], F32, name="ot_ps", tag="mk")
                nc.tensor.matmul(ot_ps, ptp_t[:, 0:DH], hf_all[:, fs],
                                 start=True, stop=False)
                nc.tensor.matmul(ot_ps, ptp_t[:, DH:128], hf_all[:, fs2],
                                 start=False, stop=False)
                nc.tensor.matmul(ot_ps, vns_t[0:1, hs],
                                 rr_all[:, rb * 128:(rb + 1) * 128],
                                 start=False, stop=True)
                nc.scalar.activation(
                    ctx_tiles[rb][DH * (h % 2):DH * (h % 2) + DH,
                                  128 * (h // 2):128 * (h // 2) + 128],
                    ot_ps, AF.Identity, bias=pb_t)

        # ---- output projection + bias per row block ----
        for rb in range(NRB):
            y_ps = psum.tile([128, D_MODEL], F32, name="y_ps", tag="mm")
            for n0, nw in ((0, 512), (512, 256)):
                for ci in range(3):
                    nc.tensor.matmul(y_ps[:, n0:n0 + nw],
                                     ctx_tiles[rb][:, ci * 128:(ci + 1) * 128],
                                     wo_t[ci][:, n0:n0 + nw],
                                     start=(ci == 0), stop=False)
                nc.tensor.matmul(y_ps[:, n0:n0 + nw], ones_row,
                                 wob8_t[0:1, n0:n0 + nw], start=False, stop=True)
            y_sb = sb2.tile([128, D_MODEL], F32, name="y_sb", tag="y_sb")
            nc.vector.tensor_copy(y_sb, y_ps)
            nc.sync.dma_start(
                ybounce[b * T + rb * 128: b * T + (rb + 1) * 128, :], y_sb)

    if single_core:
        nc.sync.dma_start(io["Y"][:, :], ybounce)
    else:
        nc.gpsimd.collective_compute(
            "ReduceScatter", ALU.add, replica_groups=[list(range(N_CORES))],
            ins=[ybounce.opt()], outs=[yrs.opt()])
        nc.sync.dma_start(io["Y"][:, :], yrs)

    for p in (sb4, sb3, sb2, sb1, psum, dpool, cpool):
        p.release()


_CACHE = {}
REPS = 1


def _build(single_core=False):
    key = ("nc_sim" if single_core else "nc") + str(REPS)
    if key in _CACHE:
        return _CACHE[key]
    nc = bacc.Bacc("TRN2", target_bir_lowering=False, debug=False,
                   num_devices=1 if single_core else N_CORES,
                   enable_asserts=False)
    io = {
        "AX16": nc.dram_tensor("AX16", [2 * B * T, D_MODEL], BF16,
                               kind="ExternalInput"),
        "WB32": nc.dram_tensor("WB32", [_blob_size(WSPEC32)], F32,
                               kind="ExternalInput"),
        "WB16": nc.dram_tensor("WB16", [_blob_size(WSPEC16)], BF16,
                               kind="ExternalInput"),
        "Y": nc.dram_tensor(
            "Y", [B * T if single_core else B * T // N_CORES, D_MODEL], F32,
            kind="ExternalOutput"),
    }
    with tile.TileContext(nc) as tc:
        _emit(tc, io, single_core=single_core, reps=REPS)
    nc.compile()
    _CACHE[key] = nc
    return nc


def _prep_in_maps(inputs):
    A = np.asarray(inputs["A"], np.float32)
    X = np.asarray(inputs["X"], np.float32)
    Wq_w = np.asarray(inputs["Wq_w"], np.float32)
    Wq_b = np.asarray(inputs["Wq_b"], np.float32)
    Wk_w = np.asarray(inputs["Wk_w"], np.float32)
    Wk_b = np.asarray(inputs["Wk_b"], np.float32)
    wedge_A = np.asarray(inputs["wedge_A"], np.float32)
    wb = np.asarray(inputs["wedge_bias"], np.float32)
    sink = np.asarray(inputs["sink_scalars"], np.float32).reshape(H_TOT)
    v_nulls = np.asarray(inputs["v_nulls"], np.float32).reshape(H_TOT, DH)
    fc_w = np.asarray(inputs["fc_w"], np.float32)
    fc_b = np.asarray(inputs["fc_b"], np.float32)
    proj_w = np.asarray(inputs["proj_w"], np.float32)
    proj_b = np.asarray(inputs["proj_b"], np.float32)
    WO = np.asarray(inputs["WO"], np.float32)
    WO_b = np.asarray(inputs["WO_b"], np.float32)

    AX = np.concatenate([A.reshape(B * T, D_MODEL),
                         X.reshape(B * T, D_MODEL)], axis=0)
    AX16 = np.ascontiguousarray(AX.astype(ml_dtypes.bfloat16))

    skew = wedge_A - wedge_A.T
    inv_freq = 1.0 / (10000.0 ** (np.arange(0, DH, 2, dtype=np.float32) / DH))
    freqs = np.arange(T, dtype=np.float32)[:, None] * inv_freq[None, :]
    cosT = np.cos(freqs).T.astype(np.float32)
    sinT = np.sin(freqs).T.astype(np.float32)
    C2 = np.concatenate([cosT, sinT], axis=0)
    S2 = np.concatenate([sinT, cosT], axis=0)
    FA = np.concatenate([fc_w[:, PERM].T, fc_b[None, :]], axis=0)
    PT = (proj_w / MLP_SCALE).T.astype(np.float32)
    PTP = np.concatenate([PT[0:128], PT[128:256]], axis=1)
    PB = proj_b[:, None]
    WOB8 = (WO_b.mean(axis=0) / N_CORES)[None, :]
    eye = np.eye(DH, dtype=np.float32)

    in_maps = []
    for c in range(N_CORES):
        h0 = c * HPC
        br = h0 // N_HEAD
        s0 = h0 % N_HEAD
        rq = np.concatenate([(h0 + h) * DH + PERM for h in range(HPC)])
        rk = np.concatenate([(s0 + h) * DH + PERM for h in range(HPC)])
        WqT = Wq_w[rq].T
        QB = Wq_b[rq].reshape(HPC, DH).T
        WkT = Wk_w[rk].T
        KB = Wk_b[rk].reshape(HPC, DH).T
        wedges = []
        for h in range(HPC):
            g = h0 + h
            S_h = skew + np.diag(wb[g])
            wedges.append(((eye + S_h.T)[PERM][:, PERM]).T)
        WEDGE = np.concatenate(wedges, axis=1)
        es = np.exp(sink[h0:h0 + HPC]).astype(np.float32)
        ESINK = np.broadcast_to(es[None, :], (128, HPC))
        VNS = (v_nulls[h0:h0 + HPC] * es[:, None]).reshape(1, HPC * DH)
        WOr = WO[br, s0 * DH:(s0 + HPC) * DH, :] / float(N_BR)
        vals = {
            "C2": C2, "S2": S2, "ESINK": ESINK,
            "VNS": VNS, "PB": PB,
            "WOB8": WOB8, "QB": QB, "KB": KB,
        }
        vals16 = {
            "WqT16": WqT, "WkT16": WkT, "WEDGE": WEDGE, "FA": FA,
            "PTP": PTP, "WOr": WOr,
        }
        wb32 = np.concatenate(
            [np.asarray(vals[n], np.float32).ravel() for n, _ in WSPEC32])
        wb16 = np.concatenate(
            [np.asarray(vals16[n], ml_dtypes.bfloat16).ravel()
             for n, _ in WSPEC16])
        in_maps.append({"AX16": AX16, "WB32": np.ascontiguousarray(wb32),
                        "WB16": np.ascontiguousarray(wb16)})
    return in_maps


def run(inputs, **kwargs):
    nc = _build()
    in_maps = _prep_in_maps(inputs)
    res = run_bass_kernel_spmd(nc, in_maps, core_ids=list(range(N_CORES)),
                               **kwargs)
    parts = [res.results[c]["Y"] for c in range(N_CORES)]
    y = np.concatenate(parts, axis=0).reshape(B, T, D_MODEL)
    return y.astype(np.float32), res


def kernel(**inputs) -> np.ndarray:
    y, _ = run(inputs)
    return y
